# revision 6
# baseline (speedup 1.0000x reference)
"""BrainModel kernel for 8 TRN2 NeuronCores (raw bass, no Tile).

Reference computation:
    gathered = x[:, idx]                              # [B, O, C]
    pre = einsum('boc,oc->bo', gathered, w_sparse) + b_sparse
    new_x = sigmoid(pre)                              # [B, O]
    q = new_x[:, -N_MOTORS:] @ w_motor.T + b_motor    # [B, A]

Only the last N_MOTORS=256 rows of idx/w_sparse/b_sparse reach q, so the
other 98720 output neurons are dead code. We shard those 256 motor
neurons across the 8 cores (32 each).

Per-core device program:
  1. two HWDGE DMAs: a small f32 "aux" tile (gather indices bitcast to
     int32 + b_sparse + b_motor/8) and a bf16 "auxw" tile (expanded
     block-sparse weights Wk usable directly as matmul lhsT + wmT).
  2. a warm-up SWDGE indirect gather (zero indices) issued BEFORE the
     idx-DMA wait: absorbs the ~1us first-SWDGE-use init (ring/scratch
     setup) that otherwise lands on the critical path.
  3. 8 SWDGE indirect DMAs gather 128 rows each (x[:, i] for fan-in
     index i) from the bf16 transposed table tbl=[N_NEURONS, B]. The HW
     DGE consumes ONE index per partition per instruction (verified: a
     [128, J] offset AP or 3D dest AP writes garbage), so 8 gathers is
     the minimum; each costs ~1.1us of Q7 descriptor emission (994ns
     fixed + ~0.7ns/desc) and they pipeline with the matmuls.
  4. 8 accumulating bf16 matmuls -> pre [32, B] in PSUM; ScalarE
     sigmoid(+b_sparse bias) -> s bf16; bf16 matmul vs wmT -> q partial
     [A, B] (+ b_motor/8 on the PSUM->SBUF copy); one HWDGE DMA out.
Host sums the 8 partials and transposes to [B, A].

Raw bass keeps every instruction at <= 1 semaphore wait (the TRN2
walrus codegen rejects multi-wait Matmult/Drain encodings) and avoids
the Tile kernel-tail drain + all-engine barrier entirely.
"""

from contextlib import ExitStack

import ml_dtypes
import numpy as np

import concourse.bass as bass
from concourse import mybir

N_NEURONS = 100000
N_MOTORS = 256
N_CONN = 32
N_ACT = 16
BATCH = 64
N_CORES = 8
M_PER_CORE = N_MOTORS // N_CORES  # 32 motor neurons per core
R = M_PER_CORE * N_CONN  # 1024 gathered x-rows per core
P = 128  # SBUF partitions
J = R // P  # 8 gather/matmul chunks

AUXC = J  # aux is idx-only: 8 int32 columns bitcast to f32
C_WMT = J * M_PER_CORE  # 256: wmT starts here in auxw
C_BS = C_WMT + N_ACT  # 272: b_sparse col (bf16)
C_BM = C_BS + 1  # 273: b_motor/8 col (bf16)
AUXWC = C_BM + 1  # 274

_CACHE: dict = {}


def _build_nc() -> bass.Bass:
    f32 = mybir.dt.float32
    bf16 = mybir.dt.bfloat16
    i32 = mybir.dt.int32
    nc = bass.Bass(enable_partition_id=False)

    tbl = nc.declare_dram_parameter("tbl", [N_NEURONS, BATCH], bf16, isOutput=False)
    aux = nc.declare_dram_parameter("aux", [P, AUXC], f32, isOutput=False)
    auxw = nc.declare_dram_parameter("auxw", [P, AUXWC], bf16, isOutput=False)
    out = nc.declare_dram_parameter("out", [N_ACT, BATCH], f32, isOutput=True)

    with ExitStack() as ctx:
        aux_sb = ctx.enter_context(nc.sbuf_tensor("aux_sb", [P, AUXC], f32))
        auxw_sb = ctx.enter_context(nc.sbuf_tensor("auxw_sb", [P, AUXWC], bf16))
        G = ctx.enter_context(nc.sbuf_tensor("G", [P, J * BATCH], bf16))
        Gw = ctx.enter_context(nc.sbuf_tensor("Gw", [P, BATCH], bf16))
        widx = ctx.enter_context(nc.sbuf_tensor("widx", [P, 1], i32))
        s_sb = ctx.enter_context(nc.sbuf_tensor("s_sb", [M_PER_CORE, BATCH], bf16))
        q_sb = ctx.enter_context(nc.sbuf_tensor("q_sb", [N_ACT, BATCH], f32))
        pre_ps = ctx.enter_context(nc.psum_tensor("pre_ps", [M_PER_CORE, BATCH], f32))
        q_ps = ctx.enter_context(nc.psum_tensor("q_ps", [N_ACT, BATCH], f32))
        isem = ctx.enter_context(nc.semaphore("isem"))
        isem2 = ctx.enter_context(nc.semaphore("isem2"))
        wsem = ctx.enter_context(nc.semaphore("wsem"))
        warmsem = ctx.enter_context(nc.semaphore("warmsem"))
        odma_sem = ctx.enter_context(nc.semaphore("odma_sem"))
        # One completion sem per gather chunk: a single shared sem would be
        # racy -- each DMA's 16 increments come from 16 independent SDMA
        # engines, so a running count can reach 16*(j+1) before chunk j has
        # fully landed.
        gdma_sems = [
            ctx.enter_context(nc.semaphore(f"gdma_sem{j}")) for j in range(J)
        ]
        pe_sem = ctx.enter_context(nc.semaphore("pe_sem"))
        act_sem = ctx.enter_context(nc.semaphore("act_sem"))
        warm_sb = ctx.enter_context(nc.sbuf_tensor("warm_sb", [1, 1], f32))
        pad_sb = ctx.enter_context(nc.sbuf_tensor("pad_sb", [1, 1], f32))
        block = ctx.enter_context(nc.Block())

        @block.sync
        def _(sync):
            # idx first (split in two: gathers 0-3 start off the smaller
            # first half), weights on their own sem (completion order of
            # separate DMAs is not guaranteed).
            sync.dma_start(out=aux_sb[:, : J // 2], in_=aux[:, : J // 2]).then_inc(
                isem, 16
            )
            sync.dma_start(out=aux_sb[:, J // 2 :], in_=aux[:, J // 2 :]).then_inc(
                isem2, 16
            )
            sync.dma_start(out=auxw_sb[:], in_=auxw[:]).then_inc(wsem, 16)
            sync.wait_ge(odma_sem, 16)

        @block.gpsimd
        def _(gpsimd):
            # Pre-wait warm-up: zero-index gather into scratch. Pays the
            # one-time SWDGE ring/scratch init + the post-dispatch stall
            # while the idx DMA is still in flight.
            gpsimd.memset(widx[:], 0)
            gpsimd.indirect_dma_start(
                out=Gw[:],
                out_offset=None,
                in_=tbl[:],
                in_offset=bass.IndirectOffsetOnAxis(ap=widx[:], axis=0),
            ).then_inc(warmsem, 16)
            gpsimd.wait_ge(isem, 16)
            # Cheap op right after the wait: the Pool sequencer has a ~1us
            # dispatch stall on the first instruction after a wait; let a
            # 1-element memset absorb it instead of the first gather.
            gpsimd.memset(pad_sb[:], 0)
            # The HW DGE consumes ONE index per partition per instruction:
            # partition p of the dest gets dest-free-size contiguous bytes
            # starting at tbl row idx[p]. So one gather per chunk j.
            for j in range(J):
                if j == J // 2:
                    # second idx half has long since landed: satisfied wait
                    gpsimd.wait_ge(isem2, 16)
                gpsimd.indirect_dma_start(
                    out=G[:, j * BATCH : (j + 1) * BATCH],
                    out_offset=None,
                    in_=tbl[:],
                    in_offset=bass.IndirectOffsetOnAxis(
                        ap=aux_sb[:, j : j + 1].bitcast(i32),
                        axis=0,
                    ),
                ).then_inc(gdma_sems[j], 16)
            gpsimd.wait_ge(warmsem, 16)

        @block.tensor
        def _(tensor):
            tensor.wait_ge(wsem, 16)
            # pre[m, b] = sum_{p,j} Wk[p, j*32+m] * x[b, idx_flat[p*J+j]]
            for j in range(J):
                tensor.wait_ge(gdma_sems[j], 16)
                mm = tensor.matmul(
                    pre_ps[:],
                    auxw_sb[:, j * M_PER_CORE : (j + 1) * M_PER_CORE],
                    G[:, j * BATCH : (j + 1) * BATCH],
                    start=(j == 0),
                    stop=(j == J - 1),
                )
            mm.then_inc(pe_sem, 1)
            tensor.wait_ge(act_sem, 1)
            # q_part[a, b] = sum_m wmT[m, a] * s[m, b]
            tensor.matmul(
                q_ps[:],
                auxw_sb[:M_PER_CORE, C_WMT : C_WMT + N_ACT],
                s_sb[:],
                start=True,
                stop=True,
            ).then_inc(pe_sem, 1)

        @block.scalar
        def _(scalar):
            # Dummy activation preloads the sigmoid LUT off the critical path
            # (the table load is ~1.3us and otherwise serializes after the
            # last matmul). Reads the already-landed aux_sb.
            scalar.wait_ge(isem, 16)
            scalar.activation(
                warm_sb[:],
                aux_sb[:1, :1],
                mybir.ActivationFunctionType.Sigmoid,
            )
            scalar.wait_ge(pe_sem, 1)
            # s = sigmoid(pre + b_sparse), bf16 out feeds the bf16 motor mm
            scalar.activation(
                s_sb[:],
                pre_ps[:],
                mybir.ActivationFunctionType.Sigmoid,
                bias=auxw_sb[:M_PER_CORE, C_BS : C_BS + 1],
            ).then_inc(act_sem, 1)
            scalar.wait_ge(pe_sem, 2)
            # q_sb = q_ps + b_motor/8 (PSUM -> SBUF)
            scalar.activation(
                q_sb[:],
                q_ps[:],
                mybir.ActivationFunctionType.Identity,
                bias=auxw_sb[:N_ACT, C_BM : C_BM + 1],
            )
            # ScalarE is HWDGE-capable: issue the output DMA right here,
            # skipping a cross-engine semaphore hop to Sync.
            scalar.dma_start(out=out[:], in_=q_sb[:]).then_inc(odma_sem, 16)

    return nc


def _get_nc() -> bass.Bass:
    if "nc" not in _CACHE:
        _CACHE["nc"] = _build_nc()
    return _CACHE["nc"]


def make_in_maps(x, idx, w_sparse, b_sparse, w_motor, b_motor):
    """Shard FULL inputs into the 8 per-core input dicts."""
    x = np.asarray(x, dtype=np.float32)
    idx_m = np.asarray(idx)[-N_MOTORS:].astype(np.int32)  # [256, 32]
    w_m = np.asarray(w_sparse, dtype=np.float32)[-N_MOTORS:]  # [256, 32]
    b_m = np.asarray(b_sparse, dtype=np.float32)[-N_MOTORS:]  # [256]
    wm = np.asarray(w_motor, dtype=np.float32)  # [16, 256]
    bm = np.asarray(b_motor, dtype=np.float32)  # [16]

    # bf16 transposed table: row i = x[:, i] (128B rows)
    xT = np.ascontiguousarray(x.T).astype(ml_dtypes.bfloat16)  # [N_NEURONS, B]

    r = np.arange(R)
    o_l, c = r // N_CONN, r % N_CONN
    p_r, j_r = r // J, r % J

    in_maps = []
    for k in range(N_CORES):
        rows = slice(k * M_PER_CORE, (k + 1) * M_PER_CORE)
        w_core = w_m[rows]  # [32, 32]

        idx_tile = np.ascontiguousarray(idx_m[rows].reshape(P, J))  # int32
        aux = idx_tile.view(np.float32)

        auxw = np.zeros((P, AUXWC), ml_dtypes.bfloat16)
        # Wk[p, j*32+m] = w[m, c] at r = p*J+j = m*32+c, else 0
        Wk = np.zeros((P, C_WMT), np.float32)
        Wk[p_r, j_r * M_PER_CORE + o_l] = w_core[o_l, c]
        auxw[:, :C_WMT] = Wk.astype(ml_dtypes.bfloat16)
        auxw[:M_PER_CORE, C_WMT:C_BS] = wm[:, rows].T.astype(ml_dtypes.bfloat16)
        auxw[:M_PER_CORE, C_BS] = b_m[rows].astype(ml_dtypes.bfloat16)
        auxw[:N_ACT, C_BM] = (bm / N_CORES).astype(ml_dtypes.bfloat16)

        in_maps.append({"tbl": xT, "aux": aux, "auxw": auxw})
    return in_maps


def combine_outputs(partials):
    """Reduce the 8 per-core [A, B] partials to the full [B, A] output."""
    q = np.sum(np.stack(partials, axis=0), axis=0, dtype=np.float64)
    return np.ascontiguousarray(q.T).astype(np.float32)


def _ensure_trace_hook_importable():
    """bass_utils' axon trace path imports antenv.axon_hooks; some containers
    ship an antenv without it. Provide a null hook so trace degrades to a
    plain run instead of crashing."""
    import os

    if not os.environ.get("BASS_TRACE"):
        return
    try:
        import antenv.axon_hooks  # noqa: F401
    except ImportError:
        import sys
        import types

        import antenv

        m = types.ModuleType("antenv.axon_hooks")
        state = {"hook": None}
        m.set_axon_ntff_profile_hook = lambda h: state.__setitem__("hook", h)
        m.get_axon_ntff_profile_hook = lambda: state["hook"]
        sys.modules["antenv.axon_hooks"] = m
        antenv.axon_hooks = m


def kernel(x, idx, w_sparse, b_sparse, w_motor, b_motor):
    from concourse.bass_utils import run_bass_kernel_spmd

    _ensure_trace_hook_importable()
    nc = _get_nc()
    in_maps = make_in_maps(x, idx, w_sparse, b_sparse, w_motor, b_motor)
    res = run_bass_kernel_spmd(nc, in_maps, core_ids=list(range(N_CORES)))
    _CACHE["last_results"] = res
    return combine_outputs([res.results[k]["out"] for k in range(N_CORES)])


# revision 7
# speedup vs baseline: 1.0069x; 1.0069x over previous
"""BrainModel kernel for 8 TRN2 NeuronCores (raw bass, no Tile).

Reference computation:
    gathered = x[:, idx]                              # [B, O, C]
    pre = einsum('boc,oc->bo', gathered, w_sparse) + b_sparse
    new_x = sigmoid(pre)                              # [B, O]
    q = new_x[:, -N_MOTORS:] @ w_motor.T + b_motor    # [B, A]

Only the last N_MOTORS=256 rows of idx/w_sparse/b_sparse reach q, so the
other 98720 output neurons are dead code. We shard those 256 motor
neurons across the 8 cores (32 each).

Per-core device program:
  1. two HWDGE DMAs: a small f32 "aux" tile (gather indices bitcast to
     int32 + b_sparse + b_motor/8) and a bf16 "auxw" tile (expanded
     block-sparse weights Wk usable directly as matmul lhsT + wmT).
  2. a warm-up SWDGE indirect gather (zero indices) issued BEFORE the
     idx-DMA wait: absorbs the ~1us first-SWDGE-use init (ring/scratch
     setup) that otherwise lands on the critical path.
  3. 8 SWDGE indirect DMAs gather 128 rows each (x[:, i] for fan-in
     index i) from the bf16 transposed table tbl=[N_NEURONS, B]. The HW
     DGE consumes ONE index per partition per instruction (verified: a
     [128, J] offset AP or 3D dest AP writes garbage), so 8 gathers is
     the minimum; each costs ~1.1us of Q7 descriptor emission (994ns
     fixed + ~0.7ns/desc) and they pipeline with the matmuls.
  4. 8 accumulating bf16 matmuls -> pre [32, B] in PSUM; ScalarE
     sigmoid(+b_sparse bias) -> s bf16; bf16 matmul vs wmT -> q partial
     [A, B] (+ b_motor/8 on the PSUM->SBUF copy); one HWDGE DMA out.
Host sums the 8 partials and transposes to [B, A].

Raw bass keeps every instruction at <= 1 semaphore wait (the TRN2
walrus codegen rejects multi-wait Matmult/Drain encodings) and avoids
the Tile kernel-tail drain + all-engine barrier entirely.
"""

from contextlib import ExitStack

import ml_dtypes
import numpy as np

import concourse.bass as bass
from concourse import mybir

N_NEURONS = 100000
N_MOTORS = 256
N_CONN = 32
N_ACT = 16
BATCH = 64
N_CORES = 8
M_PER_CORE = N_MOTORS // N_CORES  # 32 motor neurons per core
R = M_PER_CORE * N_CONN  # 1024 gathered x-rows per core
P = 128  # SBUF partitions
J = R // P  # 8 gather/matmul chunks

AUXC = J  # aux is idx-only: 8 int32 columns bitcast to f32
C_WMT = J * M_PER_CORE  # 256: wmT starts here in auxw
C_BS = C_WMT + N_ACT  # 272: b_sparse col (bf16)
C_BM = C_BS + 1  # 273: b_motor/8 col (bf16)
AUXWC = C_BM + 1  # 274

_CACHE: dict = {}


def _build_nc() -> bass.Bass:
    f32 = mybir.dt.float32
    bf16 = mybir.dt.bfloat16
    i32 = mybir.dt.int32
    nc = bass.Bass(enable_partition_id=False)

    tbl = nc.declare_dram_parameter("tbl", [N_NEURONS, BATCH], bf16, isOutput=False)
    aux = nc.declare_dram_parameter("aux", [P, AUXC], f32, isOutput=False)
    auxw = nc.declare_dram_parameter("auxw", [P, AUXWC], bf16, isOutput=False)
    out = nc.declare_dram_parameter("out", [N_ACT, BATCH], f32, isOutput=True)

    with ExitStack() as ctx:
        aux_sb = ctx.enter_context(nc.sbuf_tensor("aux_sb", [P, AUXC], f32))
        auxw_sb = ctx.enter_context(nc.sbuf_tensor("auxw_sb", [P, AUXWC], bf16))
        G = ctx.enter_context(nc.sbuf_tensor("G", [P, J * BATCH], bf16))
        Gw = ctx.enter_context(nc.sbuf_tensor("Gw", [P, BATCH], bf16))
        widx = ctx.enter_context(nc.sbuf_tensor("widx", [P, 1], i32))
        s_sb = ctx.enter_context(nc.sbuf_tensor("s_sb", [M_PER_CORE, BATCH], bf16))
        q_sb = ctx.enter_context(nc.sbuf_tensor("q_sb", [N_ACT, BATCH], f32))
        pre_ps = ctx.enter_context(nc.psum_tensor("pre_ps", [M_PER_CORE, BATCH], f32))
        q_ps = ctx.enter_context(nc.psum_tensor("q_ps", [N_ACT, BATCH], f32))
        isem = ctx.enter_context(nc.semaphore("isem"))
        wsem = ctx.enter_context(nc.semaphore("wsem"))
        warmsem = ctx.enter_context(nc.semaphore("warmsem"))
        odma_sem = ctx.enter_context(nc.semaphore("odma_sem"))
        # One completion sem per gather chunk: a single shared sem would be
        # racy -- each DMA's 16 increments come from 16 independent SDMA
        # engines, so a running count can reach 16*(j+1) before chunk j has
        # fully landed.
        gdma_sems = [
            ctx.enter_context(nc.semaphore(f"gdma_sem{j}")) for j in range(J)
        ]
        pe_sem = ctx.enter_context(nc.semaphore("pe_sem"))
        act_sem = ctx.enter_context(nc.semaphore("act_sem"))
        warm_sb = ctx.enter_context(nc.sbuf_tensor("warm_sb", [1, 1], f32))
        pad_sb = ctx.enter_context(nc.sbuf_tensor("pad_sb", [1, 1], f32))
        block = ctx.enter_context(nc.Block())

        @block.sync
        def _(sync):
            # idx + biases first (small) so the gathers start ASAP; weights on
            # their own sem (completion order of two DMAs is not guaranteed).
            sync.dma_start(out=aux_sb[:], in_=aux[:]).then_inc(isem, 16)
            sync.dma_start(out=auxw_sb[:], in_=auxw[:]).then_inc(wsem, 16)
            sync.wait_ge(odma_sem, 16)

        @block.gpsimd
        def _(gpsimd):
            # Pre-wait warm-up: zero-index gather into scratch. Pays the
            # one-time SWDGE ring/scratch init + the post-dispatch stall
            # while the idx DMA is still in flight.
            gpsimd.memset(widx[:], 0)
            gpsimd.indirect_dma_start(
                out=Gw[:],
                out_offset=None,
                in_=tbl[:],
                in_offset=bass.IndirectOffsetOnAxis(ap=widx[:], axis=0),
            ).then_inc(warmsem, 16)
            gpsimd.wait_ge(isem, 16)
            # Cheap op right after the wait: the Pool sequencer has a ~1us
            # dispatch stall on the first instruction after a wait; let a
            # 1-element memset absorb it instead of the first gather.
            gpsimd.memset(pad_sb[:], 0)
            # The HW DGE consumes ONE index per partition per instruction:
            # partition p of the dest gets dest-free-size contiguous bytes
            # starting at tbl row idx[p]. So one gather per chunk j.
            for j in range(J):
                gpsimd.indirect_dma_start(
                    out=G[:, j * BATCH : (j + 1) * BATCH],
                    out_offset=None,
                    in_=tbl[:],
                    in_offset=bass.IndirectOffsetOnAxis(
                        ap=aux_sb[:, j : j + 1].bitcast(i32),
                        axis=0,
                    ),
                ).then_inc(gdma_sems[j], 16)
            gpsimd.wait_ge(warmsem, 16)

        @block.tensor
        def _(tensor):
            tensor.wait_ge(wsem, 16)
            # pre[m, b] = sum_{p,j} Wk[p, j*32+m] * x[b, idx_flat[p*J+j]]
            for j in range(J):
                tensor.wait_ge(gdma_sems[j], 16)
                mm = tensor.matmul(
                    pre_ps[:],
                    auxw_sb[:, j * M_PER_CORE : (j + 1) * M_PER_CORE],
                    G[:, j * BATCH : (j + 1) * BATCH],
                    start=(j == 0),
                    stop=(j == J - 1),
                )
            mm.then_inc(pe_sem, 1)
            tensor.wait_ge(act_sem, 1)
            # q_part[a, b] = sum_m wmT[m, a] * s[m, b]
            tensor.matmul(
                q_ps[:],
                auxw_sb[:M_PER_CORE, C_WMT : C_WMT + N_ACT],
                s_sb[:],
                start=True,
                stop=True,
            ).then_inc(pe_sem, 1)

        @block.scalar
        def _(scalar):
            # Dummy activation preloads the sigmoid LUT off the critical path
            # (the table load is ~1.3us and otherwise serializes after the
            # last matmul). Reads the already-landed aux_sb.
            scalar.wait_ge(isem, 16)
            scalar.activation(
                warm_sb[:],
                aux_sb[:1, :1],
                mybir.ActivationFunctionType.Sigmoid,
            )
            scalar.wait_ge(pe_sem, 1)
            # s = sigmoid(pre + b_sparse), bf16 out feeds the bf16 motor mm
            scalar.activation(
                s_sb[:],
                pre_ps[:],
                mybir.ActivationFunctionType.Sigmoid,
                bias=auxw_sb[:M_PER_CORE, C_BS : C_BS + 1],
            ).then_inc(act_sem, 1)
            scalar.wait_ge(pe_sem, 2)
            # q_sb = q_ps + b_motor/8 (PSUM -> SBUF)
            scalar.activation(
                q_sb[:],
                q_ps[:],
                mybir.ActivationFunctionType.Identity,
                bias=auxw_sb[:N_ACT, C_BM : C_BM + 1],
            )
            # ScalarE is HWDGE-capable: issue the output DMA right here,
            # skipping a cross-engine semaphore hop to Sync.
            scalar.dma_start(out=out[:], in_=q_sb[:]).then_inc(odma_sem, 16)

    return nc


def _get_nc() -> bass.Bass:
    if "nc" not in _CACHE:
        _CACHE["nc"] = _build_nc()
    return _CACHE["nc"]


def make_in_maps(x, idx, w_sparse, b_sparse, w_motor, b_motor):
    """Shard FULL inputs into the 8 per-core input dicts."""
    x = np.asarray(x, dtype=np.float32)
    idx_m = np.asarray(idx)[-N_MOTORS:].astype(np.int32)  # [256, 32]
    w_m = np.asarray(w_sparse, dtype=np.float32)[-N_MOTORS:]  # [256, 32]
    b_m = np.asarray(b_sparse, dtype=np.float32)[-N_MOTORS:]  # [256]
    wm = np.asarray(w_motor, dtype=np.float32)  # [16, 256]
    bm = np.asarray(b_motor, dtype=np.float32)  # [16]

    # bf16 transposed table: row i = x[:, i] (128B rows)
    xT = np.ascontiguousarray(x.T).astype(ml_dtypes.bfloat16)  # [N_NEURONS, B]

    r = np.arange(R)
    o_l, c = r // N_CONN, r % N_CONN
    p_r, j_r = r // J, r % J

    in_maps = []
    for k in range(N_CORES):
        rows = slice(k * M_PER_CORE, (k + 1) * M_PER_CORE)
        w_core = w_m[rows]  # [32, 32]

        idx_tile = np.ascontiguousarray(idx_m[rows].reshape(P, J))  # int32
        aux = idx_tile.view(np.float32)

        auxw = np.zeros((P, AUXWC), ml_dtypes.bfloat16)
        # Wk[p, j*32+m] = w[m, c] at r = p*J+j = m*32+c, else 0
        Wk = np.zeros((P, C_WMT), np.float32)
        Wk[p_r, j_r * M_PER_CORE + o_l] = w_core[o_l, c]
        auxw[:, :C_WMT] = Wk.astype(ml_dtypes.bfloat16)
        auxw[:M_PER_CORE, C_WMT:C_BS] = wm[:, rows].T.astype(ml_dtypes.bfloat16)
        auxw[:M_PER_CORE, C_BS] = b_m[rows].astype(ml_dtypes.bfloat16)
        auxw[:N_ACT, C_BM] = (bm / N_CORES).astype(ml_dtypes.bfloat16)

        in_maps.append({"tbl": xT, "aux": aux, "auxw": auxw})
    return in_maps


def combine_outputs(partials):
    """Reduce the 8 per-core [A, B] partials to the full [B, A] output."""
    q = np.sum(np.stack(partials, axis=0), axis=0, dtype=np.float64)
    return np.ascontiguousarray(q.T).astype(np.float32)


def _ensure_trace_hook_importable():
    """bass_utils' axon trace path imports antenv.axon_hooks; some containers
    ship an antenv without it. Provide a null hook so trace degrades to a
    plain run instead of crashing."""
    import os

    if not os.environ.get("BASS_TRACE"):
        return
    try:
        import antenv.axon_hooks  # noqa: F401
    except ImportError:
        import sys
        import types

        import antenv

        m = types.ModuleType("antenv.axon_hooks")
        state = {"hook": None}
        m.set_axon_ntff_profile_hook = lambda h: state.__setitem__("hook", h)
        m.get_axon_ntff_profile_hook = lambda: state["hook"]
        sys.modules["antenv.axon_hooks"] = m
        antenv.axon_hooks = m


def kernel(x, idx, w_sparse, b_sparse, w_motor, b_motor):
    from concourse.bass_utils import run_bass_kernel_spmd

    _ensure_trace_hook_importable()
    nc = _get_nc()
    in_maps = make_in_maps(x, idx, w_sparse, b_sparse, w_motor, b_motor)
    res = run_bass_kernel_spmd(nc, in_maps, core_ids=list(range(N_CORES)))
    _CACHE["last_results"] = res
    return combine_outputs([res.results[k]["out"] for k in range(N_CORES)])


# revision 8
# speedup vs baseline: 1.0203x; 1.0133x over previous
"""BrainModel kernel for 8 TRN2 NeuronCores (raw bass, no Tile).

Reference computation:
    gathered = x[:, idx]                              # [B, O, C]
    pre = einsum('boc,oc->bo', gathered, w_sparse) + b_sparse
    new_x = sigmoid(pre)                              # [B, O]
    q = new_x[:, -N_MOTORS:] @ w_motor.T + b_motor    # [B, A]

Only the last N_MOTORS=256 rows of idx/w_sparse/b_sparse reach q, so the
other 98720 output neurons are dead code. We shard those 256 motor
neurons across the 8 cores (32 each).

Per-core device program:
  1. two HWDGE DMAs: a small f32 "aux" tile (gather indices bitcast to
     int32 + b_sparse + b_motor/8) and a bf16 "auxw" tile (expanded
     block-sparse weights Wk usable directly as matmul lhsT + wmT).
  2. a warm-up SWDGE indirect gather (zero indices) issued BEFORE the
     idx-DMA wait: absorbs the ~1us first-SWDGE-use init (ring/scratch
     setup) that otherwise lands on the critical path.
  3. 8 SWDGE indirect DMAs gather 128 rows each (x[:, i] for fan-in
     index i) from the bf16 transposed table tbl=[N_NEURONS, B]. The HW
     DGE consumes ONE index per partition per instruction (verified: a
     [128, J] offset AP or 3D dest AP writes garbage), so 8 gathers is
     the minimum; each costs ~1.1us of Q7 descriptor emission (994ns
     fixed + ~0.7ns/desc) and they pipeline with the matmuls.
  4. 8 accumulating bf16 matmuls -> pre [32, B] in PSUM; ScalarE
     sigmoid(+b_sparse bias) -> s bf16; bf16 matmul vs wmT -> q partial
     [A, B] (+ b_motor/8 on the PSUM->SBUF copy); one HWDGE DMA out.
Host sums the 8 partials and transposes to [B, A].

Raw bass keeps every instruction at <= 1 semaphore wait (the TRN2
walrus codegen rejects multi-wait Matmult/Drain encodings) and avoids
the Tile kernel-tail drain + all-engine barrier entirely.
"""

from contextlib import ExitStack

import ml_dtypes
import numpy as np

import concourse.bass as bass
from concourse import mybir

N_NEURONS = 100000
N_MOTORS = 256
N_CONN = 32
N_ACT = 16
BATCH = 64
N_CORES = 8
M_PER_CORE = N_MOTORS // N_CORES  # 32 motor neurons per core
R = M_PER_CORE * N_CONN  # 1024 gathered x-rows per core
P = 128  # SBUF partitions
J = R // P  # 8 gather/matmul chunks

AUXC = J  # aux is idx-only: 8 int32 columns bitcast to f32
C_WMT = J * M_PER_CORE  # 256: wmT starts here in auxw
C_BS = C_WMT + N_ACT  # 272: b_sparse col (bf16)
C_BM = C_BS + 1  # 273: b_motor/8 col (bf16)
AUXWC = C_BM + 1  # 274

_CACHE: dict = {}


def _build_nc() -> bass.Bass:
    f32 = mybir.dt.float32
    bf16 = mybir.dt.bfloat16
    i32 = mybir.dt.int32
    nc = bass.Bass(enable_partition_id=False)

    tbl = nc.declare_dram_parameter("tbl", [N_NEURONS, BATCH], bf16, isOutput=False)
    aux = nc.declare_dram_parameter("aux", [P, AUXC], f32, isOutput=False)
    auxw = nc.declare_dram_parameter("auxw", [P, AUXWC], bf16, isOutput=False)
    out = nc.declare_dram_parameter("out", [N_ACT, BATCH], f32, isOutput=True)

    with ExitStack() as ctx:
        aux_sb = ctx.enter_context(nc.sbuf_tensor("aux_sb", [P, AUXC], f32))
        auxw_sb = ctx.enter_context(nc.sbuf_tensor("auxw_sb", [P, AUXWC], bf16))
        G = ctx.enter_context(nc.sbuf_tensor("G", [P, J * BATCH], bf16))
        Gw = ctx.enter_context(nc.sbuf_tensor("Gw", [P, BATCH], bf16))
        widx = ctx.enter_context(nc.sbuf_tensor("widx", [P, 1], i32))
        s_sb = ctx.enter_context(nc.sbuf_tensor("s_sb", [M_PER_CORE, BATCH], bf16))
        q_sb = ctx.enter_context(nc.sbuf_tensor("q_sb", [N_ACT, BATCH], f32))
        pre_ps = ctx.enter_context(nc.psum_tensor("pre_ps", [M_PER_CORE, BATCH], f32))
        q_ps = ctx.enter_context(nc.psum_tensor("q_ps", [N_ACT, BATCH], f32))
        isem = ctx.enter_context(nc.semaphore("isem"))
        wsem = ctx.enter_context(nc.semaphore("wsem"))
        warmsem = ctx.enter_context(nc.semaphore("warmsem"))
        odma_sem = ctx.enter_context(nc.semaphore("odma_sem"))
        # One completion sem per gather chunk: a single shared sem would be
        # racy -- each DMA's 16 increments come from 16 independent SDMA
        # engines, so a running count can reach 16*(j+1) before chunk j has
        # fully landed.
        gdma_sems = [
            ctx.enter_context(nc.semaphore(f"gdma_sem{j}")) for j in range(J)
        ]
        pe_sem = ctx.enter_context(nc.semaphore("pe_sem"))
        act_sem = ctx.enter_context(nc.semaphore("act_sem"))
        warm_sb = ctx.enter_context(nc.sbuf_tensor("warm_sb", [1, 1], f32))
        pad_sb = ctx.enter_context(nc.sbuf_tensor("pad_sb", [1, 1], f32))
        block = ctx.enter_context(nc.Block())

        @block.sync
        def _(sync):
            # idx + biases first (small) so the gathers start ASAP; weights on
            # their own sem (completion order of two DMAs is not guaranteed).
            sync.dma_start(out=aux_sb[:], in_=aux[:]).then_inc(isem, 16)
            sync.dma_start(out=auxw_sb[:], in_=auxw[:]).then_inc(wsem, 16)

        @block.gpsimd
        def _(gpsimd):
            # Pre-wait warm-up: zero-index gather into scratch. Pays the
            # one-time SWDGE ring/scratch init + the post-dispatch stall
            # while the idx DMA is still in flight.
            gpsimd.memset(widx[:], 0)
            gpsimd.indirect_dma_start(
                out=Gw[:],
                out_offset=None,
                in_=tbl[:],
                in_offset=bass.IndirectOffsetOnAxis(ap=widx[:], axis=0),
            ).then_inc(warmsem, 16)
            gpsimd.wait_ge(isem, 16)
            # Cheap op right after the wait: the Pool sequencer has a ~1us
            # dispatch stall on the first instruction after a wait; let a
            # 1-element memset absorb it instead of the first gather.
            gpsimd.memset(pad_sb[:], 0)
            # The HW DGE consumes ONE index per partition per instruction:
            # partition p of the dest gets dest-free-size contiguous bytes
            # starting at tbl row idx[p]. So one gather per chunk j.
            for j in range(J):
                gpsimd.indirect_dma_start(
                    out=G[:, j * BATCH : (j + 1) * BATCH],
                    out_offset=None,
                    in_=tbl[:],
                    in_offset=bass.IndirectOffsetOnAxis(
                        ap=aux_sb[:, j : j + 1].bitcast(i32),
                        axis=0,
                    ),
                ).then_inc(gdma_sems[j], 16)
            gpsimd.wait_ge(warmsem, 16)

        @block.tensor
        def _(tensor):
            tensor.wait_ge(wsem, 16)
            # pre[m, b] = sum_{p,j} Wk[p, j*32+m] * x[b, idx_flat[p*J+j]]
            for j in range(J):
                tensor.wait_ge(gdma_sems[j], 16)
                mm = tensor.matmul(
                    pre_ps[:],
                    auxw_sb[:, j * M_PER_CORE : (j + 1) * M_PER_CORE],
                    G[:, j * BATCH : (j + 1) * BATCH],
                    start=(j == 0),
                    stop=(j == J - 1),
                )
            mm.then_inc(pe_sem, 1)
            tensor.wait_ge(act_sem, 1)
            # q_part[a, b] = sum_m wmT[m, a] * s[m, b]
            tensor.matmul(
                q_ps[:],
                auxw_sb[:M_PER_CORE, C_WMT : C_WMT + N_ACT],
                s_sb[:],
                start=True,
                stop=True,
            ).then_inc(pe_sem, 1)

        @block.scalar
        def _(scalar):
            # Dummy activation preloads the sigmoid LUT off the critical path
            # (the table load is ~1.3us and otherwise serializes after the
            # last matmul). Reads the already-landed aux_sb.
            scalar.wait_ge(isem, 16)
            scalar.activation(
                warm_sb[:],
                aux_sb[:1, :1],
                mybir.ActivationFunctionType.Sigmoid,
            )
            scalar.wait_ge(pe_sem, 1)
            # s = sigmoid(pre + b_sparse), bf16 out feeds the bf16 motor mm
            scalar.activation(
                s_sb[:],
                pre_ps[:],
                mybir.ActivationFunctionType.Sigmoid,
                bias=auxw_sb[:M_PER_CORE, C_BS : C_BS + 1],
            ).then_inc(act_sem, 1)
            scalar.wait_ge(pe_sem, 2)
            # q_sb = q_ps + b_motor/8 (PSUM -> SBUF)
            scalar.activation(
                q_sb[:],
                q_ps[:],
                mybir.ActivationFunctionType.Identity,
                bias=auxw_sb[:N_ACT, C_BM : C_BM + 1],
            )
            # ScalarE is HWDGE-capable: issue the output DMA right here,
            # skipping a cross-engine semaphore hop to Sync.
            scalar.dma_start(out=out[:], in_=q_sb[:]).then_inc(odma_sem, 16)

    return nc


def _get_nc() -> bass.Bass:
    if "nc" not in _CACHE:
        _CACHE["nc"] = _build_nc()
    return _CACHE["nc"]


def make_in_maps(x, idx, w_sparse, b_sparse, w_motor, b_motor):
    """Shard FULL inputs into the 8 per-core input dicts."""
    x = np.asarray(x, dtype=np.float32)
    idx_m = np.asarray(idx)[-N_MOTORS:].astype(np.int32)  # [256, 32]
    w_m = np.asarray(w_sparse, dtype=np.float32)[-N_MOTORS:]  # [256, 32]
    b_m = np.asarray(b_sparse, dtype=np.float32)[-N_MOTORS:]  # [256]
    wm = np.asarray(w_motor, dtype=np.float32)  # [16, 256]
    bm = np.asarray(b_motor, dtype=np.float32)  # [16]

    # bf16 transposed table: row i = x[:, i] (128B rows)
    xT = np.ascontiguousarray(x.T).astype(ml_dtypes.bfloat16)  # [N_NEURONS, B]

    in_maps = []
    for k in range(N_CORES):
        rows = slice(k * M_PER_CORE, (k + 1) * M_PER_CORE)
        w_core = w_m[rows]  # [32, 32]

        # Sort slots by gather index: chunk j covers the j-th contiguous
        # ascending range, so each gather's 128 HBM reads cluster in a
        # ~1.6MB window in address order (row-buffer friendly receipt).
        idx_flat = idx_m[rows].reshape(R)
        w_flat = w_core.reshape(R)
        o_flat = np.arange(R) // N_CONN
        order = np.argsort(idx_flat, kind="stable")
        rank_p = np.arange(R) % P
        rank_j = np.arange(R) // P

        idx_tile = np.zeros((P, J), np.int32)
        idx_tile[rank_p, rank_j] = idx_flat[order]
        aux = np.ascontiguousarray(idx_tile).view(np.float32)

        auxw = np.zeros((P, AUXWC), ml_dtypes.bfloat16)
        # Wk[p, j*32+m] = weight of the rank r = j*128+p slot
        Wk = np.zeros((P, C_WMT), np.float32)
        Wk[rank_p, rank_j * M_PER_CORE + o_flat[order]] = w_flat[order]
        auxw[:, :C_WMT] = Wk.astype(ml_dtypes.bfloat16)
        auxw[:M_PER_CORE, C_WMT:C_BS] = wm[:, rows].T.astype(ml_dtypes.bfloat16)
        auxw[:M_PER_CORE, C_BS] = b_m[rows].astype(ml_dtypes.bfloat16)
        auxw[:N_ACT, C_BM] = (bm / N_CORES).astype(ml_dtypes.bfloat16)

        in_maps.append({"tbl": xT, "aux": aux, "auxw": auxw})
    return in_maps


def combine_outputs(partials):
    """Reduce the 8 per-core [A, B] partials to the full [B, A] output."""
    q = np.sum(np.stack(partials, axis=0), axis=0, dtype=np.float64)
    return np.ascontiguousarray(q.T).astype(np.float32)


def _ensure_trace_hook_importable():
    """bass_utils' axon trace path imports antenv.axon_hooks; some containers
    ship an antenv without it. Provide a null hook so trace degrades to a
    plain run instead of crashing."""
    import os

    if not os.environ.get("BASS_TRACE"):
        return
    try:
        import antenv.axon_hooks  # noqa: F401
    except ImportError:
        import sys
        import types

        import antenv

        m = types.ModuleType("antenv.axon_hooks")
        state = {"hook": None}
        m.set_axon_ntff_profile_hook = lambda h: state.__setitem__("hook", h)
        m.get_axon_ntff_profile_hook = lambda: state["hook"]
        sys.modules["antenv.axon_hooks"] = m
        antenv.axon_hooks = m


def kernel(x, idx, w_sparse, b_sparse, w_motor, b_motor):
    from concourse.bass_utils import run_bass_kernel_spmd

    _ensure_trace_hook_importable()
    nc = _get_nc()
    in_maps = make_in_maps(x, idx, w_sparse, b_sparse, w_motor, b_motor)
    res = run_bass_kernel_spmd(nc, in_maps, core_ids=list(range(N_CORES)))
    _CACHE["last_results"] = res
    return combine_outputs([res.results[k]["out"] for k in range(N_CORES)])


# revision 9
# speedup vs baseline: 1.0511x; 1.0302x over previous
"""BrainModel kernel for 8 TRN2 NeuronCores (raw bass, no Tile).

Reference computation:
    gathered = x[:, idx]                              # [B, O, C]
    pre = einsum('boc,oc->bo', gathered, w_sparse) + b_sparse
    new_x = sigmoid(pre)                              # [B, O]
    q = new_x[:, -N_MOTORS:] @ w_motor.T + b_motor    # [B, A]

Only the last N_MOTORS=256 rows of idx/w_sparse/b_sparse reach q, so the
other 98720 output neurons are dead code. We shard those 256 motor
neurons across the 8 cores (32 each).

Per-core device program:
  1. two HWDGE DMAs: a small f32 "aux" tile (gather indices bitcast to
     int32 + b_sparse + b_motor/8) and a bf16 "auxw" tile (expanded
     block-sparse weights Wk usable directly as matmul lhsT + wmT).
  2. a warm-up SWDGE indirect gather (zero indices) issued BEFORE the
     idx-DMA wait: absorbs the ~1us first-SWDGE-use init (ring/scratch
     setup) that otherwise lands on the critical path.
  3. 8 SWDGE indirect DMAs gather 128 rows each (x[:, i] for fan-in
     index i) from the bf16 transposed table tbl=[N_NEURONS, B]. The HW
     DGE consumes ONE index per partition per instruction (verified: a
     [128, J] offset AP or 3D dest AP writes garbage), so 8 gathers is
     the minimum; each costs ~1.1us of Q7 descriptor emission (994ns
     fixed + ~0.7ns/desc) and they pipeline with the matmuls.
  4. 8 accumulating bf16 matmuls -> pre [32, B] in PSUM; ScalarE
     sigmoid(+b_sparse bias) -> s bf16; bf16 matmul vs wmT -> q partial
     [A, B] (+ b_motor/8 on the PSUM->SBUF copy); one HWDGE DMA out.
Host sums the 8 partials and transposes to [B, A].

Raw bass keeps every instruction at <= 1 semaphore wait (the TRN2
walrus codegen rejects multi-wait Matmult/Drain encodings) and avoids
the Tile kernel-tail drain + all-engine barrier entirely.
"""

from contextlib import ExitStack

import ml_dtypes
import numpy as np

import concourse.bass as bass
from concourse import mybir

N_NEURONS = 100000
N_MOTORS = 256
N_CONN = 32
N_ACT = 16
BATCH = 64
N_CORES = 8
M_PER_CORE = N_MOTORS // N_CORES  # 32 motor neurons per core
R = M_PER_CORE * N_CONN  # 1024 gathered x-rows per core
P = 128  # SBUF partitions
J = R // P  # 8 gather/matmul chunks

AUXC = J  # aux is idx-only: 8 int32 columns bitcast to f32
C_WMT = J * M_PER_CORE  # 256: wmT starts here in auxw
C_BS = C_WMT + N_ACT  # 272: b_sparse col (bf16)
C_BM = C_BS + 1  # 273: b_motor/8 col (bf16)
AUXWC = C_BM + 1  # 274

_CACHE: dict = {}


def _build_nc() -> bass.Bass:
    f32 = mybir.dt.float32
    bf16 = mybir.dt.bfloat16
    i32 = mybir.dt.int32
    nc = bass.Bass(enable_partition_id=False)

    tbl = nc.declare_dram_parameter("tbl", [N_NEURONS, BATCH], bf16, isOutput=False)
    aux = nc.declare_dram_parameter("aux", [P, AUXC], f32, isOutput=False)
    auxw = nc.declare_dram_parameter("auxw", [P, AUXWC], bf16, isOutput=False)
    out = nc.declare_dram_parameter("out", [N_ACT, BATCH], f32, isOutput=True)

    with ExitStack() as ctx:
        aux_sb = ctx.enter_context(nc.sbuf_tensor("aux_sb", [P, AUXC], f32))
        auxw_sb = ctx.enter_context(nc.sbuf_tensor("auxw_sb", [P, AUXWC], bf16))
        G = ctx.enter_context(nc.sbuf_tensor("G", [P, J * BATCH], bf16))
        Gw = ctx.enter_context(nc.sbuf_tensor("Gw", [P, BATCH], bf16))
        widx = ctx.enter_context(nc.sbuf_tensor("widx", [P, 1], i32))
        s_sb = ctx.enter_context(nc.sbuf_tensor("s_sb", [M_PER_CORE, BATCH], bf16))
        q_sb = ctx.enter_context(nc.sbuf_tensor("q_sb", [N_ACT, BATCH], f32))
        pre_ps = ctx.enter_context(nc.psum_tensor("pre_ps", [M_PER_CORE, BATCH], f32))
        q_ps = ctx.enter_context(nc.psum_tensor("q_ps", [N_ACT, BATCH], f32))
        isem = ctx.enter_context(nc.semaphore("isem"))
        wsem = ctx.enter_context(nc.semaphore("wsem"))
        warmsem = ctx.enter_context(nc.semaphore("warmsem"))
        odma_sem = ctx.enter_context(nc.semaphore("odma_sem"))
        # One completion sem per gather chunk: a single shared sem would be
        # racy -- each DMA's 16 increments come from 16 independent SDMA
        # engines, so a running count can reach 16*(j+1) before chunk j has
        # fully landed.
        gdma_sems = [
            ctx.enter_context(nc.semaphore(f"gdma_sem{j}")) for j in range(J)
        ]
        pe_sem = ctx.enter_context(nc.semaphore("pe_sem"))
        act_sem = ctx.enter_context(nc.semaphore("act_sem"))
        warm_sb = ctx.enter_context(nc.sbuf_tensor("warm_sb", [1, 1], f32))
        pad_sb = ctx.enter_context(nc.sbuf_tensor("pad_sb", [1, 1], f32))
        block = ctx.enter_context(nc.Block())

        @block.sync
        def _(sync):
            # idx + biases first (small) so the gathers start ASAP; weights on
            # their own sem (completion order of two DMAs is not guaranteed).
            sync.dma_start(out=aux_sb[:], in_=aux[:]).then_inc(isem, 16)
            sync.dma_start(out=auxw_sb[:], in_=auxw[:]).then_inc(wsem, 16)

        @block.gpsimd
        def _(gpsimd):
            # Pre-wait warm-up: zero-index gather into scratch. Pays the
            # one-time SWDGE ring/scratch init + the post-dispatch stall
            # while the idx DMA is still in flight.
            gpsimd.memset(widx[:], 0)
            gpsimd.indirect_dma_start(
                out=Gw[:],
                out_offset=None,
                in_=tbl[:],
                in_offset=bass.IndirectOffsetOnAxis(ap=widx[:], axis=0),
            ).then_inc(warmsem, 16)
            gpsimd.wait_ge(isem, 16)
            # The HW DGE consumes ONE index per partition per instruction:
            # partition p of the dest gets dest-free-size contiguous bytes
            # starting at tbl row idx[p]. So one gather per chunk j.
            for j in range(J):
                gpsimd.indirect_dma_start(
                    out=G[:, j * BATCH : (j + 1) * BATCH],
                    out_offset=None,
                    in_=tbl[:],
                    in_offset=bass.IndirectOffsetOnAxis(
                        ap=aux_sb[:, j : j + 1].bitcast(i32),
                        axis=0,
                    ),
                ).then_inc(gdma_sems[j], 16)
            gpsimd.wait_ge(warmsem, 16)

        @block.tensor
        def _(tensor):
            tensor.wait_ge(wsem, 16)
            # pre[m, b] = sum_{p,j} Wk[p, j*32+m] * x[b, idx_flat[p*J+j]]
            for j in range(J):
                tensor.wait_ge(gdma_sems[j], 16)
                mm = tensor.matmul(
                    pre_ps[:],
                    auxw_sb[:, j * M_PER_CORE : (j + 1) * M_PER_CORE],
                    G[:, j * BATCH : (j + 1) * BATCH],
                    start=(j == 0),
                    stop=(j == J - 1),
                )
            mm.then_inc(pe_sem, 1)
            tensor.wait_ge(act_sem, 1)
            # q_part[a, b] = sum_m wmT[m, a] * s[m, b]
            tensor.matmul(
                q_ps[:],
                auxw_sb[:M_PER_CORE, C_WMT : C_WMT + N_ACT],
                s_sb[:],
                start=True,
                stop=True,
            ).then_inc(pe_sem, 1)

        @block.scalar
        def _(scalar):
            # Dummy activation preloads the sigmoid LUT off the critical path
            # (the table load is ~1.3us and otherwise serializes after the
            # last matmul). Reads the already-landed aux_sb.
            scalar.wait_ge(isem, 16)
            scalar.activation(
                warm_sb[:],
                aux_sb[:1, :1],
                mybir.ActivationFunctionType.Sigmoid,
            )
            scalar.wait_ge(pe_sem, 1)
            # s = sigmoid(pre + b_sparse), bf16 out feeds the bf16 motor mm
            scalar.activation(
                s_sb[:],
                pre_ps[:],
                mybir.ActivationFunctionType.Sigmoid,
                bias=auxw_sb[:M_PER_CORE, C_BS : C_BS + 1],
            ).then_inc(act_sem, 1)
            scalar.wait_ge(pe_sem, 2)
            # q_sb = q_ps + b_motor/8 (PSUM -> SBUF)
            scalar.activation(
                q_sb[:],
                q_ps[:],
                mybir.ActivationFunctionType.Identity,
                bias=auxw_sb[:N_ACT, C_BM : C_BM + 1],
            )
            # ScalarE is HWDGE-capable: issue the output DMA right here,
            # skipping a cross-engine semaphore hop to Sync.
            scalar.dma_start(out=out[:], in_=q_sb[:]).then_inc(odma_sem, 16)

    return nc


def _get_nc() -> bass.Bass:
    if "nc" not in _CACHE:
        _CACHE["nc"] = _build_nc()
    return _CACHE["nc"]


def make_in_maps(x, idx, w_sparse, b_sparse, w_motor, b_motor):
    """Shard FULL inputs into the 8 per-core input dicts."""
    x = np.asarray(x, dtype=np.float32)
    idx_m = np.asarray(idx)[-N_MOTORS:].astype(np.int32)  # [256, 32]
    w_m = np.asarray(w_sparse, dtype=np.float32)[-N_MOTORS:]  # [256, 32]
    b_m = np.asarray(b_sparse, dtype=np.float32)[-N_MOTORS:]  # [256]
    wm = np.asarray(w_motor, dtype=np.float32)  # [16, 256]
    bm = np.asarray(b_motor, dtype=np.float32)  # [16]

    # bf16 transposed table: row i = x[:, i] (128B rows)
    xT = np.ascontiguousarray(x.T).astype(ml_dtypes.bfloat16)  # [N_NEURONS, B]

    in_maps = []
    for k in range(N_CORES):
        rows = slice(k * M_PER_CORE, (k + 1) * M_PER_CORE)
        w_core = w_m[rows]  # [32, 32]

        # Sort slots by gather index: chunk j covers the j-th contiguous
        # ascending range, so each gather's 128 HBM reads cluster in a
        # ~1.6MB window in address order (row-buffer friendly receipt).
        idx_flat = idx_m[rows].reshape(R)
        w_flat = w_core.reshape(R)
        o_flat = np.arange(R) // N_CONN
        order = np.argsort(idx_flat, kind="stable")
        rank_p = np.arange(R) % P
        rank_j = np.arange(R) // P

        idx_tile = np.zeros((P, J), np.int32)
        idx_tile[rank_p, rank_j] = idx_flat[order]
        aux = np.ascontiguousarray(idx_tile).view(np.float32)

        auxw = np.zeros((P, AUXWC), ml_dtypes.bfloat16)
        # Wk[p, j*32+m] = weight of the rank r = j*128+p slot
        Wk = np.zeros((P, C_WMT), np.float32)
        Wk[rank_p, rank_j * M_PER_CORE + o_flat[order]] = w_flat[order]
        auxw[:, :C_WMT] = Wk.astype(ml_dtypes.bfloat16)
        auxw[:M_PER_CORE, C_WMT:C_BS] = wm[:, rows].T.astype(ml_dtypes.bfloat16)
        auxw[:M_PER_CORE, C_BS] = b_m[rows].astype(ml_dtypes.bfloat16)
        auxw[:N_ACT, C_BM] = (bm / N_CORES).astype(ml_dtypes.bfloat16)

        in_maps.append({"tbl": xT, "aux": aux, "auxw": auxw})
    return in_maps


def combine_outputs(partials):
    """Reduce the 8 per-core [A, B] partials to the full [B, A] output."""
    q = np.sum(np.stack(partials, axis=0), axis=0, dtype=np.float64)
    return np.ascontiguousarray(q.T).astype(np.float32)


def _ensure_trace_hook_importable():
    """bass_utils' axon trace path imports antenv.axon_hooks; some containers
    ship an antenv without it. Provide a null hook so trace degrades to a
    plain run instead of crashing."""
    import os

    if not os.environ.get("BASS_TRACE"):
        return
    try:
        import antenv.axon_hooks  # noqa: F401
    except ImportError:
        import sys
        import types

        import antenv

        m = types.ModuleType("antenv.axon_hooks")
        state = {"hook": None}
        m.set_axon_ntff_profile_hook = lambda h: state.__setitem__("hook", h)
        m.get_axon_ntff_profile_hook = lambda: state["hook"]
        sys.modules["antenv.axon_hooks"] = m
        antenv.axon_hooks = m


def kernel(x, idx, w_sparse, b_sparse, w_motor, b_motor):
    from concourse.bass_utils import run_bass_kernel_spmd

    _ensure_trace_hook_importable()
    nc = _get_nc()
    in_maps = make_in_maps(x, idx, w_sparse, b_sparse, w_motor, b_motor)
    res = run_bass_kernel_spmd(nc, in_maps, core_ids=list(range(N_CORES)))
    _CACHE["last_results"] = res
    return combine_outputs([res.results[k]["out"] for k in range(N_CORES)])
